# revision 29
# baseline (speedup 1.0000x reference)
"""Distributed causal attention block (QKV + RoPE + SDPA + Wo) on 8 TRN2 cores.

Sharding: tensor-parallel over heads (2 heads/core). Each core:
  phase 1: weight-stationary transposed QKV: q^T/k^T = Wqkv_c @ x^T streamed
           from host-pretransposed x^T (bf16); RoPE applied in the [e, t]
           layout with per-head even/odd partition split; v computed DIRECTLY
           in [t, e] layout (x-chunk-stationary matmuls), no transposes
  phase 2: causal attention per (batch, head) with TRANSPOSED scores
           s^T[k, q], q-chunk-major: each 512-token q-chunk computes its
           k-blocks in order, exp's them in bank-pair-wide ACT ops into
           per-pair S tiles, and immediately feeds PV PSUM-accumulation
           (pair-lagged) plus softmax-sum accumulation via all-ones matmuls
           into a packed PSUM bank pair; 1/sum = exp(-ln(sum)) on ACT
  phase 3: AllGather attention outputs (bounced per-chunk across DMA queues)
           -> Wo e-slice, interleaved with phase 2(b=1); batch-1's v is
           computed between AG(b0) and phase 2(b=1) as PE filler under the
           collective's latency
Host concatenates the 8 e-slices.

The q/k rows of Wqkv (and cos/sin tables) are permuted head-major
even/odd on the host; attention scores are invariant to a shared
permutation of the head dim of Q and K.
"""
import numpy as np
import ml_dtypes
import bass_rust
import concourse.bass as bass
import concourse.mybir as mybir
from concourse.tile import TileContext

B, L, D, H = 2, 2048, 2048, 16
HD = 128
N_CORES = 8
HPC = H // N_CORES          # heads per core = 2
ES = HPC * HD               # 256 = e-slice width per core
T = B * L                   # 4096 tokens total
P = 128
SCALE = 1.0 / float(np.sqrt(HD))
FP = mybir.dt.float32
BF = mybir.dt.bfloat16

N_TT = T // P               # 32 global t-tiles
N_LT = L // P               # 16 t-tiles per batch
N_DT = D // P               # 16 d-tiles

# attention-out AllGather pieces per batch, in units of 512-t q-chunks
AG_PIECES = {0: [(0, 2), (2, 4)], 1: [(0, 2), (2, 3), (3, 4)]}


def piece_of(b, qc):
    for (c0, c1) in AG_PIECES[b]:
        if c0 <= qc < c1:
            return (c0, c1)
    raise AssertionError


def chunk_blocks(qc):
    """k-blocks of q-chunk qc: (kt, off, w) in kt order.
    off = column offset within the chunk (fully-masked prefix), w = width."""
    return [(kt, max(0, kt * 128 - qc * 512), 512 - max(0, kt * 128 - qc * 512))
            for kt in range(4 * qc + 4)]


def split_multi_waits(nc):
    """This walrus build allows 1 sync wait per instruction (2 for
    EventSemaphore). Tile attaches more on some instructions (tail drain,
    collective-adjacent DMAs); hoist the extras onto same-engine NoOps."""
    for f in nc.m.functions:
        for bb in f.blocks:
            new_insts = []
            changed = False
            for ins in bb.instructions:
                si = ins.sync_info
                cap = 2 if type(ins).__name__ == "InstEventSemaphore" else 1
                if si is not None and len(si.on_wait) > cap:
                    waits = list(si.on_wait)
                    for k, w in enumerate(waits[cap:]):
                        new_insts.append(mybir.InstNoOp(
                            name=f"{ins.name}-wsplit{k}", ins=[], outs=[],
                            engine=ins.engine,
                            sync_info=bass_rust.SyncInfo(on_wait=[w], on_update=[]),
                        ))
                    ins.sync_info = bass_rust.SyncInfo(
                        on_wait=waits[:cap], on_update=list(si.on_update))
                    changed = True
                new_insts.append(ins)
            if changed:
                bb.instructions.clear()
                for i2 in new_insts:
                    bb.add_instruction(i2)


def build(fix_waits=True, dummy_cc=True):
    nc = bass.Bass()
    xT = nc.declare_dram_parameter("xT", [D, T], BF, isOutput=False)
    wqkvT = nc.declare_dram_parameter("wqkvT", [D, 3 * ES], BF, isOutput=False)
    # per-head stacked trig tables: rows 0:64 = even-col table, 64:128 = odd
    cc_p = [nc.declare_dram_parameter(f"cc{h}", [P, L], BF, isOutput=False)
            for h in range(HPC)]
    ss_p = [nc.declare_dram_parameter(f"ss{h}", [P, L], BF, isOutput=False)
            for h in range(HPC)]
    woT = nc.declare_dram_parameter("woT", [D, ES], BF, isOutput=False)
    out = nc.declare_dram_parameter("out", [ES, T], BF, isOutput=True)

    o_bounce, ag_o = {}, {}
    for b, pieces in AG_PIECES.items():
        for (c0, c1) in pieces:
            w = (c1 - c0) * 512
            o_bounce[(b, c0)] = nc.dram_tensor(f"o_bounce{b}_{c0}", [ES, w], BF)
            ag_o[(b, c0)] = nc.dram_tensor(f"ag_o{b}_{c0}", [N_CORES * ES, w], BF,
                                           addr_space="Shared")
    rg = [list(range(N_CORES))]
    if dummy_cc:
        dummy_in = nc.dram_tensor("dummy_in", [1, 256], BF)
        dummy_out = nc.dram_tensor("dummy_ag", [N_CORES, 256], BF,
                                   addr_space="Shared")

    with TileContext(nc, pool_alloc_mode="queue") as tc:
        with (
            tc.tile_pool(name="const", bufs=1) as const_pool,
            tc.tile_pool(name="resident", bufs=1) as res_pool,
            tc.tile_pool(name="wo", bufs=1) as wo_pool,
            tc.tile_pool(name="pS", bufs=8) as pS,
            tc.tile_pool(name="pAcc", bufs=2) as pAcc,
            tc.tile_pool(name="p2ob", bufs=3) as p2ob,
        ):
            if dummy_cc:
                # skew-absorbing tiny collective: aligns the 8 cores while
                # phase 1 computes, so the real AllGathers don't eat the skew
                zt = const_pool.tile([1, 256], BF, name="zt")
                nc.gpsimd.memset(zt[:, :], 0.0)
                nc.gpsimd.dma_start(out=dummy_in[:, :], in_=zt[:, :])
                nc.gpsimd.collective_compute(
                    "AllGather", mybir.AluOpType.bypass,
                    ins=[dummy_in[:]], outs=[dummy_out[:]],
                    replica_groups=rg)

            ones = const_pool.tile([P, P], BF, name="ones")
            nc.gpsimd.memset(ones[:, :], 1.0)
            tri = const_pool.tile([P, P], BF, name="tri")
            nc.gpsimd.memset(tri[:, :], 1.0)
            nc.gpsimd.affine_select(
                out=tri[:, :], in_=tri[:, :],
                compare_op=mybir.AluOpType.is_ge, fill=0.0, base=0,
                pattern=[[1, P]], channel_multiplier=-1)

            # resident through phases 1-2
            qt_sb = res_pool.tile([P, HPC * T], BF, name="qt_sb")   # [hd', h*T+t]
            kt_sb = res_pool.tile([P, HPC * T], BF, name="kt_sb")
            v_sb = res_pool.tile([P, N_TT * ES], BF, name="v_sb")   # [t%128, tt*ES+e]
            woT_sb = wo_pool.tile([P, N_DT * ES], BF, name="woT_sb")

            # ---------------- phase-2 helpers ----------------
            def phase2_chunk(b, qc, psW, psSm, psO):
                blks = chunk_blocks(qc)
                pairs = [tuple(blks[i:i + 2]) for i in range(0, len(blks), 2)]
                nkt = 4 * qc + 4
                state = {}

                def pv_pair(i, h):
                    # PV + softmax-sum accumulation, both on PE (sums via
                    # all-ones matmuls into the packed PSUM bank pair);
                    # the two ones-mms share their stationary tile
                    Sp = state[(i, h)]
                    scol = 0
                    for (kt, off, w) in pairs[i]:
                        nc.tensor.matmul(
                            state["sm"][:, h * 512 + off:(h + 1) * 512],
                            ones[:, :], Sp[:, scol:scol + w],
                            start=(kt == 0), stop=(kt == nkt - 1))
                        scol += w
                    scol = 0
                    for (kt, off, w) in pairs[i]:
                        nc.tensor.matmul(
                            state[("o", h)][:, off:],
                            v_sb[:, (b * N_LT + kt) * ES + h * HD:
                                 (b * N_LT + kt) * ES + (h + 1) * HD],
                            Sp[:, scol:scol + w],
                            start=(kt == 0), stop=(kt == nkt - 1))
                        scol += w

                for i, pr in enumerate(pairs):
                    for h in range(HPC):
                        qoff = h * T + b * L
                        wtot = sum(blk[2] for blk in pr)
                        sp = psW.tile([P, 1024], FP, name="sp", tag="w")
                        Sp = pS.tile([P, 1024], BF, name="Sp", tag="S")
                        state[(i, h)] = Sp
                        spo = 0
                        for (kt, off, w) in pr:
                            nc.tensor.matmul(
                                sp[:, spo:spo + w],
                                kt_sb[:, qoff + kt * P:qoff + (kt + 1) * P],
                                qt_sb[:, qoff + qc * 512 + off:
                                      qoff + (qc + 1) * 512],
                                start=True, stop=True)
                            spo += w
                        nc.scalar.activation(
                            Sp[:, 0:wtot], sp[:, 0:wtot],
                            mybir.ActivationFunctionType.Exp, scale=SCALE)
                        # diagonal blocks: zero the masked (k>q) triangle
                        scol = 0
                        for (kt, off, w) in pr:
                            if kt >= 4 * qc:
                                nc.vector.tensor_tensor(
                                    Sp[:, scol:scol + P], Sp[:, scol:scol + P],
                                    tri[:, :], op=mybir.AluOpType.mult)
                            scol += w
                    if i == 1:
                        # lazy-allocate the chunk accumulators so their
                        # WAR waits don't gate the first score matmuls
                        state["sm"] = psSm.tile([P, 1024], FP, name="sm",
                                                tag="sm")
                        for h in range(HPC):
                            state[("o", h)] = psO.tile([P, 512], FP,
                                                       name=f"o{h}",
                                                       tag=f"o{h}")
                    if i > 0:
                        for h in range(HPC):
                            pv_pair(i - 1, h)
                for h in range(HPC):
                    pv_pair(len(pairs) - 1, h)

                # finalize: 1/sum = exp(-ln(sum)) on ACT (merged across
                # heads), rescale drain, per-chunk bounce DMA
                lsm = pAcc.tile([P, 1024], FP, name="lsm", tag="lsm",
                                bufs=1)
                nc.scalar.activation(lsm[:, :], state["sm"][:, :],
                                     mybir.ActivationFunctionType.Ln)
                rec = pAcc.tile([P, 1024], FP, name="rec", tag="rec",
                                bufs=1)
                nc.scalar.activation(rec[:, :], lsm[:, :],
                                     mybir.ActivationFunctionType.Exp,
                                     scale=-1.0)
                (c0, c1) = piece_of(b, qc)
                obc = p2ob.tile([P, 1024], BF, name="obc", tag="ob")
                for h in range(HPC):
                    nc.vector.tensor_tensor(
                        obc[:, h * 512:(h + 1) * 512],
                        state[("o", h)][:, :], rec[:, h * 512:(h + 1) * 512],
                        op=mybir.AluOpType.mult)
                    for bh in range(2):
                        nc.sync.dma_start(
                            out=o_bounce[(b, c0)][h * HD:(h + 1) * HD,
                                                  (qc - c0) * 512 + bh * 256:
                                                  (qc - c0) * 512 +
                                                  (bh + 1) * 256],
                            in_=obc[:, h * 512 + bh * 256:
                                    h * 512 + (bh + 1) * 256])

            def ag_fire(b, c0):
                nc.gpsimd.collective_compute(
                    "AllGather", mybir.AluOpType.bypass,
                    ins=[o_bounce[(b, c0)][:]],
                    outs=[ag_o[(b, c0)][:]],
                    replica_groups=rg)

            # ---------------- phase 1 + block 1 ----------------
            with (
                tc.tile_pool(name="wq", bufs=1) as wq_pool,
                tc.tile_pool(name="xt", bufs=1) as xt_pool,
            ):
                wt_sb = wq_pool.tile([P, N_DT * 3 * ES], BF, name="wt_sb")
                cc_sb = [wq_pool.tile([P, L], BF, name=f"cc{h}_sb")
                         for h in range(HPC)]
                ss_sb = [wq_pool.tile([P, L], BF, name=f"ss{h}_sb")
                         for h in range(HPC)]
                xt_sb = xt_pool.tile([P, N_DT * 2048], BF, name="xt_sb")

                # DMA priority: x^T th0 tiles + weights interleaved, then trig
                for dt in range(N_DT):
                    if dt == 0:
                        for c in range(4):
                            nc.sync.dma_start(
                                out=xt_sb[:, c * 512:(c + 1) * 512],
                                in_=xT[0:P, c * 512:(c + 1) * 512])
                    else:
                        nc.sync.dma_start(
                            out=xt_sb[:, dt * 2048:(dt + 1) * 2048],
                            in_=xT[dt * P:(dt + 1) * P, 0:2048])
                    nc.sync.dma_start(
                        out=wt_sb[:, dt * 3 * ES:(dt + 1) * 3 * ES],
                        in_=wqkvT[dt * P:(dt + 1) * P, :])
                    if dt == 3:
                        nc.sync.dma_start(out=cc_sb[0][:, :],
                                          in_=cc_p[0][:, :])
                        nc.sync.dma_start(out=ss_sb[0][:, :],
                                          in_=ss_p[0][:, :])
                nc.sync.dma_start(out=cc_sb[1][:, :], in_=cc_p[1][:, :])
                nc.sync.dma_start(out=ss_sb[1][:, :], in_=ss_p[1][:, :])

                def v_direct(psum_tile, th, tt0, ntt, drain_eng):
                    """v[t, e] tiles for global t-tiles tt0..tt0+ntt-1 via
                    x-chunk-stationary matmuls; drain to v_sb."""
                    for tl in range(ntt):
                        tcol = (tt0 - th * N_LT) * P + tl * P
                        for dt in range(N_DT):
                            nc.tensor.matmul(
                                psum_tile[:, tl * ES:(tl + 1) * ES],
                                xt_sb[:, dt * 2048 + tcol:
                                      dt * 2048 + tcol + P],
                                wt_sb[:, dt * 3 * ES + 4 * P:
                                      dt * 3 * ES + 6 * P],
                                start=(dt == 0), stop=(dt == N_DT - 1))
                    drain_eng(
                        v_sb[:, tt0 * ES:(tt0 + ntt) * ES],
                        psum_tile[:, 0:ntt * ES])

                with tc.tile_pool(name="psG", bufs=2, space="PSUM") as psG:
                    with tc.tile_pool(name="rsc", bufs=1) as rsc_pool:
                        def rope_drain(gp, dst, h, th):
                            # stage the PSUM slab to SBUF via ACT (full-128
                            # copy, no partition shift): DVE PSUM reads stall
                            # concurrent PE PSUM writes, ACT reads do not
                            cc, ss = cc_sb[h], ss_sb[h]
                            for cH in range(2):
                                gc = slice(cH * 1024, (cH + 1) * 1024)
                                dcol = slice(h * T + th * 2048 + cH * 1024,
                                             h * T + th * 2048 +
                                             (cH + 1) * 1024)
                                eo = rsc_pool.tile([P, 1024], BF, name="eo",
                                                   tag="eo", bufs=2)
                                nc.scalar.copy(eo[:, :], gp[:, gc])
                                e_ps, o_ps = eo[0:64, :], eo[64:128, :]
                                t1 = rsc_pool.tile([64, 1024], BF, name="t1",
                                                   tag="t1")
                                t2 = rsc_pool.tile([64, 1024], BF, name="t2",
                                                   tag="t2")
                                nc.vector.tensor_tensor(
                                    t1[:, :], e_ps, cc[0:64, gc],
                                    op=mybir.AluOpType.mult)
                                nc.vector.tensor_tensor(
                                    t2[:, :], o_ps, ss[64:128, gc],
                                    op=mybir.AluOpType.mult)
                                nc.vector.tensor_tensor(
                                    dst[0:64, dcol], t1[:, :], t2[:, :],
                                    op=mybir.AluOpType.subtract)
                                t3 = rsc_pool.tile([64, 1024], BF, name="t3",
                                                   tag="t1")
                                t4 = rsc_pool.tile([64, 1024], BF, name="t4",
                                                   tag="t2")
                                nc.vector.tensor_tensor(
                                    t3[:, :], o_ps, cc[64:128, gc],
                                    op=mybir.AluOpType.mult)
                                nc.vector.tensor_tensor(
                                    t4[:, :], e_ps, ss[0:64, gc],
                                    op=mybir.AluOpType.mult)
                                nc.vector.tensor_tensor(
                                    dst[64:128, dcol], t3[:, :], t4[:, :],
                                    op=mybir.AluOpType.add)

                        def qk_group(th, ebi, reload_xt):
                            gp = psG.tile([P, 2048], FP, name="gp", tag="gp")
                            for dt in range(N_DT):
                                lhsT = wt_sb[:, dt * 3 * ES + ebi * P:
                                             dt * 3 * ES + (ebi + 1) * P]
                                for c in range(4):
                                    nc.tensor.matmul(
                                        gp[:, c * 512:(c + 1) * 512], lhsT,
                                        xt_sb[:, dt * 2048 + c * 512:
                                              dt * 2048 + (c + 1) * 512],
                                        start=(dt == 0), stop=(dt == N_DT - 1))
                                if reload_xt:
                                    nc.sync.dma_start(
                                        out=xt_sb[:, dt * 2048:(dt + 1) * 2048],
                                        in_=xT[dt * P:(dt + 1) * P, 2048:4096])
                            if ebi < 2:
                                rope_drain(gp, qt_sb, ebi, th)
                            else:
                                rope_drain(gp, kt_sb, ebi - 2, th)

                        # th0: q0 first (tolerates the DMA ramp), then v,
                        # then q1/k0/k1; xt reloads th1 during the last group
                        qk_group(0, 0, reload_xt=False)
                        for g in range(2):
                            gv = psG.tile([P, 2048], FP, name="gv", tag="gp")
                            v_direct(gv, 0, g * 8, 8, nc.scalar.copy)
                        for i, ebi in enumerate([1, 2, 3]):
                            qk_group(0, ebi, reload_xt=(i == 2))
                        # th1: q/k, then half of v(b1) so the final psG
                        # drain is a cheap copy instead of a RoPE chain
                        for ebi in range(4):
                            qk_group(1, ebi, reload_xt=False)
                        gv = psG.tile([P, 2048], FP, name="gv", tag="gp")
                        v_direct(gv, 1, 16, 8, nc.scalar.copy)

                for dt in range(N_DT):
                    nc.sync.dma_start(out=woT_sb[:, dt * ES:(dt + 1) * ES],
                                      in_=woT[dt * P:(dt + 1) * P, :])

                # ---- block 1: phase2(b=0); v(b=1) runs under AG(b0) ----
                with (
                    tc.tile_pool(name="psW1", bufs=2, space="PSUM") as psW1,
                    tc.tile_pool(name="psSm1", bufs=1, space="PSUM") as psSm1,
                    tc.tile_pool(name="psO1", bufs=1, space="PSUM") as psO1,
                ):
                    phase2_chunk(0, 0, psW1, psSm1, psO1)
                    phase2_chunk(0, 1, psW1, psSm1, psO1)
                    ag_fire(0, 0)
                    phase2_chunk(0, 2, psW1, psSm1, psO1)
                    phase2_chunk(0, 3, psW1, psSm1, psO1)
                    ag_fire(0, 2)
                    for vi in range(2):         # rest of v(b1): AG filler
                        gv = psW1.tile([P, 1024], FP, name="gv", tag="w")
                        v_direct(gv, 1, 24 + vi * 4, 4, nc.vector.tensor_copy)

            # ---- block 2: phase2(b=1) with Wo pieces interleaved ----
            with (
                tc.tile_pool(name="p3x", bufs=2) as p3x,
                tc.tile_pool(name="p3o", bufs=2) as p3o,
                tc.tile_pool(name="psW2", bufs=2, space="PSUM") as psW2,
                tc.tile_pool(name="psSm2", bufs=1, space="PSUM") as psSm2,
                tc.tile_pool(name="psO2", bufs=1, space="PSUM") as psO2,
            ):
                def p3_load(b, c0, tch):
                    ot = p3x.tile([P, N_DT * 512], BF, name="ot", tag="ot")
                    for dt in range(N_DT):
                        nc.sync.dma_start(
                            out=ot[:, dt * 512:(dt + 1) * 512],
                            in_=ag_o[(b, c0)][dt * P:(dt + 1) * P,
                                              (tch - c0) * 512:
                                              (tch - c0 + 1) * 512])
                    return ot

                def p3_mm(b, tch, ot):
                    t0 = b * L + tch * 512
                    for et in range(HPC):
                        f_ps = psW2.tile([P, 1024], FP, name="f_ps", tag="w")
                        for dt in range(N_DT):
                            nc.tensor.matmul(
                                f_ps[:, 0:512],
                                woT_sb[:, dt * ES + et * P:
                                       dt * ES + (et + 1) * P],
                                ot[:, dt * 512:(dt + 1) * 512],
                                start=(dt == 0), stop=(dt == N_DT - 1))
                        f_sb = p3o.tile([P, 512], BF, name="f_sb", tag="f")
                        nc.vector.tensor_copy(f_sb[:, :], f_ps[:, 0:512])
                        nc.sync.dma_start(
                            out=out[et * P:(et + 1) * P, t0:t0 + 512],
                            in_=f_sb[:, :])

                ot00 = p3_load(0, 0, 0)
                ot01 = p3_load(0, 0, 1)
                phase2_chunk(1, 0, psW2, psSm2, psO2)
                p3_mm(0, 0, ot00)
                ot02 = p3_load(0, 2, 2)
                phase2_chunk(1, 1, psW2, psSm2, psO2)
                ag_fire(1, 0)
                p3_mm(0, 1, ot01)
                ot03 = p3_load(0, 2, 3)
                phase2_chunk(1, 2, psW2, psSm2, psO2)
                ag_fire(1, 2)
                p3_mm(0, 2, ot02)
                p3_mm(0, 3, ot03)
                phase2_chunk(1, 3, psW2, psSm2, psO2)
                ag_fire(1, 3)
                for tch in (0, 1):
                    ot = p3_load(1, 0, tch)
                    p3_mm(1, tch, ot)
                ot = p3_load(1, 2, 2)
                p3_mm(1, 2, ot)
                ot = p3_load(1, 3, 3)
                p3_mm(1, 3, ot)

    if fix_waits:
        split_multi_waits(nc)
    return nc


def make_in_maps(x, cos, sin, Wqkv, Wo):
    bf = ml_dtypes.bfloat16
    xT_full = np.ascontiguousarray(
        np.asarray(x).reshape(T, D).T).astype(bf)
    # q/k row permutation: head-major, evens then odds
    perm = []
    for h in range(HPC):
        perm.extend(h * HD + 2 * np.arange(64))
        perm.extend(h * HD + 2 * np.arange(64) + 1)
    perm = np.asarray(perm)
    in_maps = []
    cosA, sinA = np.asarray(cos), np.asarray(sin)
    for c in range(N_CORES):
        cols = slice(c * ES, (c + 1) * ES)
        wq = Wqkv[c * ES:(c + 1) * ES, :][perm]
        wk = Wqkv[D + c * ES: D + (c + 1) * ES, :][perm]
        wv = Wqkv[2 * D + c * ES: 2 * D + (c + 1) * ES, :]
        w_c = np.concatenate([wq, wk, wv], axis=0)
        m = {
            "xT": xT_full,
            "wqkvT": np.ascontiguousarray(w_c.T.astype(bf)),
            "woT": np.ascontiguousarray(Wo[cols, :].T.astype(bf)),
        }
        for h in range(HPC):
            base = c * ES + h * HD
            ce = cosA[:, base + 2 * np.arange(64)].T      # [64, L]
            co = cosA[:, base + 2 * np.arange(64) + 1].T
            se = sinA[:, base + 2 * np.arange(64)].T
            so = sinA[:, base + 2 * np.arange(64) + 1].T
            m[f"cc{h}"] = np.ascontiguousarray(
                np.concatenate([ce, co], axis=0)).astype(bf)
            # [ss_o; ss_e] so staged-SBUF rope products align bases
            m[f"ss{h}"] = np.ascontiguousarray(
                np.concatenate([so, se], axis=0)).astype(bf)
        in_maps.append(m)
    return in_maps


def gather_out(res):
    pieces = [np.asarray(res.results[c]["out"]).astype(np.float32).T
              for c in range(N_CORES)]
    return np.concatenate(pieces, axis=1).reshape(B, L, D)


_cache = {}


def kernel(x, cos, sin, Wqkv, Wo):
    from concourse.bass_utils import run_bass_kernel_spmd
    x = np.asarray(x, dtype=np.float32)
    cos = np.asarray(cos, dtype=np.float32)
    sin = np.asarray(sin, dtype=np.float32)
    Wqkv = np.asarray(Wqkv, dtype=np.float32)
    Wo = np.asarray(Wo, dtype=np.float32)
    if "nc" not in _cache:
        _cache["nc"] = build()
    nc = _cache["nc"]
    in_maps = make_in_maps(x, cos, sin, Wqkv, Wo)
    res = run_bass_kernel_spmd(nc, in_maps, core_ids=list(range(N_CORES)))
    return gather_out(res)
